# revision 7
# baseline (speedup 1.0000x reference)
"""Malvar-He-Cutler demosaic on 8 Trainium2 NeuronCores (v2).

kernel(**inputs) takes the FULL inputs (x int32 (4096,6144), kernels
(4,1,5,5) fp32) and returns the FULL (4096,6144,3) int32 output.

Row sharding: each core gets a 512-row band (+4 halo rows, reflect
padding done host-side). In-core: 5 overlapping chunks of 104 output
rows (r0 = 0,102,204,306,408) x 6 column-chunks of 1024 px.

v2 layout: input and output tiles are row-parity DEINTERLEAVED with
64-aligned parity blocks (compute engines require partition bases in
{0,32,64,96}): itile partitions [0:54] = even window rows (w=2..106
then w=0), [64:118] = odd (w=3..107 then w=1); partitions 54..63 hold
junk covered by zero lhsT columns. otile/psum: [0:52] = even output
rows, [64:116] = odd.

Passthrough mosaic values never touch the TensorEngine: only the 2/3
of outputs needing interpolation go through matmuls, packed as 4 psum
groups per (chunk, cc) -- each group stacks two half-planes (M=116)
sharing the same stride-2 rhs columns:

  grp0 (even px): G@even-rows (K0) + R@odd-rows (K2)
  grp1 (even px): B@even-rows (K3) + B@odd-rows (K1)
  grp2 (odd  px): B@even-rows (K2) + G@odd-rows (K0)
  grp3 (odd  px): R@even-rows (K1) + R@odd-rows (K3)

All matmul operands are bf16 (exact for the dyadic MHC taps; ~2^-9
relative rounding on x, far under the 2e-2 gate). Banded lhsT encode
the 5 vertical taps permuted to the deinterleaved layout; 5
accumulating matmuls (horizontal taps) per group.

Drains: one dual-op scalar_tensor_tensor per half-plane family on DVE
(clip low+high + int32 cast + stride-6 channel-interleaved write; 6
ops per chunk-cc). Passthrough: 4 Copy activations on the otherwise
idle Scalar engine. All DMA issue on gpsimd (fast DGE). Even/odd
output rows stored with separate row-strided DMAs.

_split_waits post-pass: this container's walrus accepts only ONE
semaphore wait per instruction, so excess Tile-emitted waits are
hoisted onto preceding same-engine NOPs (sequencer order preserves
semantics).
"""

import sys

import numpy as np

sys.path.insert(0, "/opt/trn_rl_repo")

import ml_dtypes

H, W = 4096, 6144
NCORES = 8
RB = H // NCORES          # 512 output rows per core
CR = 104                  # output rows per chunk (uniform, overlapping)
HR = CR // 2              # 52 rows per parity half
KR = CR + 4               # 108 window rows per chunk
ODD = 64                  # partition base of the odd-parity block
MM = ODD + HR             # 116 = psum/otile partitions (incl. dead 52..63)
KK = ODD + KR // 2        # 118 = itile partitions
CW = 1024                 # output px per column-chunk
NCC = W // CW             # 6 column-chunks
R0S = (0, 102, 204, 306, 408)   # chunk starts (rows 510..511 overlap)

# (col_parity, (half0_kernel, half0_chan), (half1_kernel, half1_chan))
GRPDEF = (
    (0, (0, 1), (2, 0)),
    (0, (3, 2), (1, 2)),
    (1, (2, 2), (0, 1)),
    (1, (1, 0), (3, 0)),
)
# passthrough: (col_parity, chan, row_parity)
PDEF = ((0, 0, 0), (0, 1, 1), (1, 1, 0), (1, 2, 1))


def _perm(w: int) -> int:
    """Window row w -> deinterleaved itile partition."""
    if w % 2 == 0:
        return 53 if w == 0 else w // 2 - 1
    return ODD + 53 if w == 1 else ODD + (w - 3) // 2


def _build_weights(kernels: np.ndarray) -> np.ndarray:
    """Deinterleave-permuted banded lhsT blocks, (128, 20*MM) bf16."""
    K = kernels[:, 0].astype(np.float32)  # (4,5,5)
    wts = np.zeros((128, 20 * MM), np.float32)
    for g, (_cp, h0, h1) in enumerate(GRPDEF):
        for dxi in range(5):
            blk = (g * 5 + dxi) * MM
            for half, (ki, _ch) in enumerate((h0, h1)):
                for mp in range(HR):
                    col = blk + half * ODD + mp
                    l = 2 * mp + half
                    for dyi in range(5):
                        v = K[ki, dyi, dxi]
                        if v != 0.0:
                            wts[_perm(l + dyi), col] = v
    return wts.astype(ml_dtypes.bfloat16)


def _split_waits(nc, maxw=1):
    """Hoist excess semaphore waits onto preceding same-engine NOPs."""
    import concourse.mybir as mybir

    nsplit = 0
    for f in nc.m.functions:
        for b in f.blocks:
            new = []
            for inst in list(b.instructions):
                si = inst.sync_info
                ow = list(si.on_wait) if si and si.on_wait else []
                if len(ow) > maxw:
                    for wx in ow[:-maxw]:
                        new.append(mybir.InstNoOp(
                            name=inst.name + f"-w{nsplit}",
                            sync_info=mybir.SyncInfo(on_wait=[wx], on_update=[]),
                            engine=inst.engine,
                            bass_nofuse=True,
                        ))
                        nsplit += 1
                    si.on_wait = ow[-maxw:]
                new.append(inst)
            b.instructions = new
    return nsplit


def _build_bass():
    import contextlib

    import concourse.bass as bass
    import concourse.mybir as mybir
    import concourse.tile as tile

    bf16 = mybir.dt.bfloat16
    f32 = mybir.dt.float32
    i32 = mybir.dt.int32
    NP = 512                  # psum cols = output px per (cc, parity)
    OB = CW * 3               # otile cols (3072)

    nc = bass.Bass()
    xb = nc.declare_dram_parameter("xb", [RB + 4, W + 4], bf16, isOutput=False)
    wts = nc.declare_dram_parameter("wts", [128, 20 * MM], bf16, isOutput=False)
    out = nc.declare_dram_parameter("out", [RB, W * 3], i32, isOutput=True)

    with contextlib.ExitStack() as ctx:
        tc = ctx.enter_context(tile.TileContext(nc))
        wpool = ctx.enter_context(tc.tile_pool(name="wpool", bufs=1))
        inpool = ctx.enter_context(tc.tile_pool(name="inpool", bufs=1))
        opool = ctx.enter_context(tc.tile_pool(name="opool", bufs=4))
        pspool = ctx.enter_context(tc.tile_pool(name="pspool", bufs=2,
                                                space="PSUM"))

        wtile = wpool.tile([128, 20 * MM], bf16)
        nc.gpsimd.dma_start(wtile[:], wts[:])
        climit = wpool.tile([128, NP], f32)
        nc.gpsimd.memset(climit[:], 16777215.0)

        # deinterleaved input tiles, all prefetched up front.  Chunk 0 is
        # issued on gpsimd (fast DGE, nothing else queued yet); the rest
        # go on the otherwise-idle Sync queue so they never serialize
        # behind output stores.  Main loads are split into column halves
        # so each chunk transfers on 4 DMA engines in parallel.
        itiles = []
        HW2 = (W + 4) // 2
        for g, r0 in enumerate(R0S):
            eng = nc.gpsimd if g == 0 else nc.sync
            it = inpool.tile([KK, W + 4], bf16, tag=f"it{g}", name=f"it{g}")
            for c0, c1 in ((0, HW2), (HW2, W + 4)):
                eng.dma_start(it[0:53, c0:c1],
                              xb[r0 + 2 : r0 + KR - 1 : 2, c0:c1])
                eng.dma_start(it[64:117, c0:c1],
                              xb[r0 + 3 : r0 + KR : 2, c0:c1])
            eng.dma_start(it[53:54, :], xb[r0 : r0 + 1, :])
            # partitions 54..63: zero-weight junk; fill with finite data
            eng.dma_start(it[54:64, :], xb[r0 + 2 : r0 + 22 : 2, :])
            eng.dma_start(it[117:118, :], xb[r0 + 1 : r0 + 2, :])
            itiles.append(it)

        for g, r0 in enumerate(R0S):
            it = itiles[g]
            for cc in range(NCC):
                otile = opool.tile([MM, OB], i32, tag="otile")
                for gi, (cp, h0, h1) in enumerate(GRPDEF):
                    ptile = pspool.tile([MM, NP], f32, tag=f"ps{gi}")
                    for dxi in range(5):
                        blk = (gi * 5 + dxi) * MM
                        c0 = CW * cc + cp + dxi
                        nc.tensor.matmul(
                            ptile[:, :],
                            wtile[:KK, blk : blk + MM],
                            it[:KK, c0 : c0 + 2 * NP - 1 : 2],
                            start=(dxi == 0),
                            stop=(dxi == 4),
                        )
                    if h0[1] == h1[1]:  # same channel: one fused drain
                        base = 3 * cp + h0[1]
                        nc.vector.scalar_tensor_tensor(
                            otile[:, base : OB : 6],
                            ptile[:, :], 0.0, climit[:MM, :],
                            op0=mybir.AluOpType.max, op1=mybir.AluOpType.min,
                        )
                    else:
                        for half, (_ki, ch) in enumerate((h0, h1)):
                            base = 3 * cp + ch
                            p0 = half * ODD
                            nc.vector.scalar_tensor_tensor(
                                otile[p0 : p0 + HR, base : OB : 6],
                                ptile[p0 : p0 + HR, :], 0.0,
                                climit[p0 : p0 + HR, :],
                                op0=mybir.AluOpType.max,
                                op1=mybir.AluOpType.min,
                            )
                # passthrough fills on the Scalar engine
                for cp, ch, rp in PDEF:
                    base = 3 * cp + ch
                    s0 = rp * ODD
                    c0 = 2 + cp + CW * cc
                    nc.scalar.activation(
                        otile[s0 : s0 + HR, base : OB : 6],
                        it[s0 : s0 + HR, c0 : c0 + 2 * NP - 1 : 2],
                        mybir.ActivationFunctionType.Copy,
                    )
                # store: even rows then odd rows (row-strided in HBM);
                # the very last otile is split column-wise to cut the tail
                ob0 = OB * cc
                nsp = 4 if (g == len(R0S) - 1 and cc == NCC - 1) else 1
                step = OB // nsp
                for sp in range(nsp):
                    o0 = sp * step
                    nc.gpsimd.dma_start(
                        out[r0 : r0 + CR : 2, ob0 + o0 : ob0 + o0 + step],
                        otile[0:HR, o0 : o0 + step])
                    nc.gpsimd.dma_start(
                        out[r0 + 1 : r0 + CR : 2, ob0 + o0 : ob0 + o0 + step],
                        otile[ODD : ODD + HR, o0 : o0 + step])
    _split_waits(nc)
    return nc


_BASS_CACHE = {}


def _get_nc():
    if "nc" not in _BASS_CACHE:
        _BASS_CACHE["nc"] = _build_bass()
    return _BASS_CACHE["nc"]


def _prepare(x: np.ndarray, kernels: np.ndarray):
    x = np.asarray(x)
    kernels = np.asarray(kernels)
    assert x.shape == (H, W) and x.dtype == np.int32

    xp = np.pad(x, 2, mode="reflect").astype(ml_dtypes.bfloat16)
    wts = _build_weights(kernels)
    in_maps = []
    for c in range(NCORES):
        band = np.ascontiguousarray(xp[c * RB : c * RB + RB + 4, :])
        in_maps.append({"xb": band, "wts": wts})
    return in_maps


def _finish(res) -> np.ndarray:
    parts = [res.results[c]["out"] for c in range(NCORES)]
    full = np.concatenate(parts, axis=0)  # (H, W*3)
    return full.reshape(H, W, 3).astype(np.int32, copy=False)


def kernel(x: np.ndarray, kernels: np.ndarray) -> np.ndarray:
    from concourse.bass_utils import run_bass_kernel_spmd

    in_maps = _prepare(x, kernels)
    nc = _get_nc()
    res = run_bass_kernel_spmd(nc, in_maps, core_ids=list(range(NCORES)))
    return _finish(res)


# revision 8
# speedup vs baseline: 1.2887x; 1.2887x over previous
"""Malvar-He-Cutler demosaic on 8 Trainium2 NeuronCores (v2).

kernel(**inputs) takes the FULL inputs (x int32 (4096,6144), kernels
(4,1,5,5) fp32) and returns the FULL (4096,6144,3) int32 output.

Row sharding: each core gets a 512-row band (+4 halo rows, reflect
padding done host-side). In-core: 5 overlapping chunks of 104 output
rows (r0 = 0,102,204,306,408) x 6 column-chunks of 1024 px.

v2 layout: input and output tiles are row-parity DEINTERLEAVED with
64-aligned parity blocks (compute engines require partition bases in
{0,32,64,96}): itile partitions [0:54] = even window rows (w=2..106
then w=0), [64:118] = odd (w=3..107 then w=1); partitions 54..63 hold
junk covered by zero lhsT columns. otile/psum: [0:52] = even output
rows, [64:116] = odd.

Passthrough mosaic values never touch the TensorEngine: only the 2/3
of outputs needing interpolation go through matmuls, packed as 4 psum
groups per (chunk, cc) -- each group stacks two half-planes (M=116)
sharing the same stride-2 rhs columns:

  grp0 (even px): G@even-rows (K0) + R@odd-rows (K2)
  grp1 (even px): B@even-rows (K3) + B@odd-rows (K1)
  grp2 (odd  px): B@even-rows (K2) + G@odd-rows (K0)
  grp3 (odd  px): R@even-rows (K1) + R@odd-rows (K3)

All matmul operands are bf16 (exact for the dyadic MHC taps; ~2^-9
relative rounding on x, far under the 2e-2 gate). Banded lhsT encode
the 5 vertical taps permuted to the deinterleaved layout; 5
accumulating matmuls (horizontal taps) per group.

Drains: one dual-op scalar_tensor_tensor per half-plane family on DVE
(clip low+high + int32 cast + stride-6 channel-interleaved write; 6
ops per chunk-cc). Passthrough: 4 Copy activations on the otherwise
idle Scalar engine. All DMA issue on gpsimd (fast DGE). Even/odd
output rows stored with separate row-strided DMAs.

_split_waits post-pass: this container's walrus accepts only ONE
semaphore wait per instruction, so excess Tile-emitted waits are
hoisted onto preceding same-engine NOPs (sequencer order preserves
semantics).
"""

import sys

import numpy as np

sys.path.insert(0, "/opt/trn_rl_repo")

import ml_dtypes

H, W = 4096, 6144
NCORES = 8
RB = H // NCORES          # 512 output rows per core
CR = 104                  # output rows per chunk (uniform, overlapping)
HR = CR // 2              # 52 rows per parity half
KR = CR + 4               # 108 window rows per chunk
ODD = 64                  # partition base of the odd-parity block
MM = ODD + HR             # 116 = psum/otile partitions (incl. dead 52..63)
KK = ODD + KR // 2        # 118 = itile partitions
CW = 1024                 # output px per column-chunk
NCC = W // CW             # 6 column-chunks
R0S = (0, 102, 204, 306, 408)   # chunk starts (rows 510..511 overlap)

# (col_parity, (half0_kernel, half0_chan), (half1_kernel, half1_chan))
GRPDEF = (
    (0, (0, 1), (2, 0)),
    (0, (3, 2), (1, 2)),
    (1, (2, 2), (0, 1)),
    (1, (1, 0), (3, 0)),
)
# passthrough: (col_parity, chan, row_parity)
PDEF = ((0, 0, 0), (0, 1, 1), (1, 1, 0), (1, 2, 1))


def _perm(w: int) -> int:
    """Window row w -> deinterleaved itile partition."""
    if w % 2 == 0:
        return 53 if w == 0 else w // 2 - 1
    return ODD + 53 if w == 1 else ODD + (w - 3) // 2


def _build_weights(kernels: np.ndarray) -> np.ndarray:
    """Deinterleave-permuted banded lhsT blocks, (128, 20*MM) bf16."""
    K = kernels[:, 0].astype(np.float32)  # (4,5,5)
    wts = np.zeros((128, 20 * MM), np.float32)
    for g, (_cp, h0, h1) in enumerate(GRPDEF):
        for dxi in range(5):
            blk = (g * 5 + dxi) * MM
            for half, (ki, _ch) in enumerate((h0, h1)):
                for mp in range(HR):
                    col = blk + half * ODD + mp
                    l = 2 * mp + half
                    for dyi in range(5):
                        v = K[ki, dyi, dxi]
                        if v != 0.0:
                            wts[_perm(l + dyi), col] = v
    return wts.astype(ml_dtypes.bfloat16)


def _split_waits(nc, maxw=1):
    """Hoist excess semaphore waits onto preceding same-engine NOPs."""
    import concourse.mybir as mybir

    nsplit = 0
    for f in nc.m.functions:
        for b in f.blocks:
            new = []
            for inst in list(b.instructions):
                si = inst.sync_info
                ow = list(si.on_wait) if si and si.on_wait else []
                if len(ow) > maxw:
                    for wx in ow[:-maxw]:
                        new.append(mybir.InstNoOp(
                            name=inst.name + f"-w{nsplit}",
                            sync_info=mybir.SyncInfo(on_wait=[wx], on_update=[]),
                            engine=inst.engine,
                            bass_nofuse=True,
                        ))
                        nsplit += 1
                    si.on_wait = ow[-maxw:]
                new.append(inst)
            b.instructions = new
    return nsplit


def _build_bass():
    import contextlib

    import concourse.bass as bass
    import concourse.mybir as mybir
    import concourse.tile as tile

    bf16 = mybir.dt.bfloat16
    f32 = mybir.dt.float32
    i32 = mybir.dt.int32
    NP = 512                  # psum cols = output px per (cc, parity)
    OB = CW * 3               # otile cols (3072)

    nc = bass.Bass()
    xb = nc.declare_dram_parameter("xb", [RB + 4, W + 4], bf16, isOutput=False)
    wts = nc.declare_dram_parameter("wts", [128, 20 * MM], bf16, isOutput=False)
    out = nc.declare_dram_parameter("out", [RB, W * 3], i32, isOutput=True)

    with contextlib.ExitStack() as ctx:
        tc = ctx.enter_context(tile.TileContext(nc))
        wpool = ctx.enter_context(tc.tile_pool(name="wpool", bufs=1))
        inpool = ctx.enter_context(tc.tile_pool(name="inpool", bufs=1))
        opool = ctx.enter_context(tc.tile_pool(name="opool", bufs=4))
        pspool = ctx.enter_context(tc.tile_pool(name="pspool", bufs=2,
                                                space="PSUM"))

        wtile = wpool.tile([128, 20 * MM], bf16)
        nc.gpsimd.dma_start(wtile[:], wts[:])
        climit = wpool.tile([128, NP], f32)
        nc.gpsimd.memset(climit[:], 16777215.0)

        # deinterleaved input tiles.  All DMA issue stays on gpsimd's
        # fast DGE queue; the runtime round-robins DMA instructions onto
        # DMA engines, so main loads are split into column quarters to
        # balance transfer time across engines.  Chunks 0-1 are issued
        # up front; chunk g+2 is issued inside chunk g's loop so input
        # issue never head-of-line-blocks the output stores.
        itiles = []
        QW = (W + 4) // 4

        def load_chunk(g):
            r0 = R0S[g]
            it = inpool.tile([KK, W + 4], bf16, tag=f"it{g}", name=f"it{g}")
            for q in range(4):
                c0, c1 = q * QW, (q + 1) * QW
                nc.gpsimd.dma_start(it[0:53, c0:c1],
                                    xb[r0 + 2 : r0 + KR - 1 : 2, c0:c1])
                nc.gpsimd.dma_start(it[64:117, c0:c1],
                                    xb[r0 + 3 : r0 + KR : 2, c0:c1])
            nc.gpsimd.dma_start(it[53:54, :], xb[r0 : r0 + 1, :])
            # partitions 54..63: zero-weight junk; fill with finite data
            nc.gpsimd.dma_start(it[54:64, :], xb[r0 + 2 : r0 + 22 : 2, :])
            nc.gpsimd.dma_start(it[117:118, :], xb[r0 + 1 : r0 + 2, :])
            itiles.append(it)

        load_chunk(0)
        load_chunk(1)

        for g, r0 in enumerate(R0S):
            it = itiles[g]
            for cc in range(NCC):
                if cc == 0 and g + 2 < len(R0S):
                    load_chunk(g + 2)
                otile = opool.tile([MM, OB], i32, tag="otile")
                for gi, (cp, h0, h1) in enumerate(GRPDEF):
                    ptile = pspool.tile([MM, NP], f32, tag=f"ps{gi}")
                    for dxi in range(5):
                        blk = (gi * 5 + dxi) * MM
                        c0 = CW * cc + cp + dxi
                        nc.tensor.matmul(
                            ptile[:, :],
                            wtile[:KK, blk : blk + MM],
                            it[:KK, c0 : c0 + 2 * NP - 1 : 2],
                            start=(dxi == 0),
                            stop=(dxi == 4),
                        )
                    if h0[1] == h1[1]:  # same channel: one fused drain
                        base = 3 * cp + h0[1]
                        nc.vector.scalar_tensor_tensor(
                            otile[:, base : OB : 6],
                            ptile[:, :], 0.0, climit[:MM, :],
                            op0=mybir.AluOpType.max, op1=mybir.AluOpType.min,
                        )
                    else:
                        for half, (_ki, ch) in enumerate((h0, h1)):
                            base = 3 * cp + ch
                            p0 = half * ODD
                            nc.vector.scalar_tensor_tensor(
                                otile[p0 : p0 + HR, base : OB : 6],
                                ptile[p0 : p0 + HR, :], 0.0,
                                climit[p0 : p0 + HR, :],
                                op0=mybir.AluOpType.max,
                                op1=mybir.AluOpType.min,
                            )
                # passthrough fills on the Scalar engine
                for cp, ch, rp in PDEF:
                    base = 3 * cp + ch
                    s0 = rp * ODD
                    c0 = 2 + cp + CW * cc
                    nc.scalar.activation(
                        otile[s0 : s0 + HR, base : OB : 6],
                        it[s0 : s0 + HR, c0 : c0 + 2 * NP - 1 : 2],
                        mybir.ActivationFunctionType.Copy,
                    )
                # store: even rows then odd rows (row-strided in HBM);
                # the very last otile is split column-wise to cut the tail
                ob0 = OB * cc
                nsp = 4 if (g == len(R0S) - 1 and cc == NCC - 1) else 1
                step = OB // nsp
                for sp in range(nsp):
                    o0 = sp * step
                    nc.gpsimd.dma_start(
                        out[r0 : r0 + CR : 2, ob0 + o0 : ob0 + o0 + step],
                        otile[0:HR, o0 : o0 + step])
                    nc.gpsimd.dma_start(
                        out[r0 + 1 : r0 + CR : 2, ob0 + o0 : ob0 + o0 + step],
                        otile[ODD : ODD + HR, o0 : o0 + step])
    _split_waits(nc)
    return nc


_BASS_CACHE = {}


def _get_nc():
    if "nc" not in _BASS_CACHE:
        _BASS_CACHE["nc"] = _build_bass()
    return _BASS_CACHE["nc"]


def _prepare(x: np.ndarray, kernels: np.ndarray):
    x = np.asarray(x)
    kernels = np.asarray(kernels)
    assert x.shape == (H, W) and x.dtype == np.int32

    xp = np.pad(x, 2, mode="reflect").astype(ml_dtypes.bfloat16)
    wts = _build_weights(kernels)
    in_maps = []
    for c in range(NCORES):
        band = np.ascontiguousarray(xp[c * RB : c * RB + RB + 4, :])
        in_maps.append({"xb": band, "wts": wts})
    return in_maps


def _finish(res) -> np.ndarray:
    parts = [res.results[c]["out"] for c in range(NCORES)]
    full = np.concatenate(parts, axis=0)  # (H, W*3)
    return full.reshape(H, W, 3).astype(np.int32, copy=False)


def kernel(x: np.ndarray, kernels: np.ndarray) -> np.ndarray:
    from concourse.bass_utils import run_bass_kernel_spmd

    in_maps = _prepare(x, kernels)
    nc = _get_nc()
    res = run_bass_kernel_spmd(nc, in_maps, core_ids=list(range(NCORES)))
    return _finish(res)


# revision 10
# speedup vs baseline: 1.4681x; 1.1392x over previous
"""Malvar-He-Cutler demosaic on 8 Trainium2 NeuronCores (v2).

kernel(**inputs) takes the FULL inputs (x int32 (4096,6144), kernels
(4,1,5,5) fp32) and returns the FULL (4096,6144,3) int32 output.

Row sharding: each core gets a 512-row band (+4 halo rows, reflect
padding done host-side). In-core: 5 overlapping chunks of 104 output
rows (r0 = 0,102,204,306,408) x 6 column-chunks of 1024 px.

v2 layout: input and output tiles are row-parity DEINTERLEAVED with
64-aligned parity blocks (compute engines require partition bases in
{0,32,64,96}): itile partitions [0:54] = even window rows (w=2..106
then w=0), [64:118] = odd (w=3..107 then w=1); partitions 54..63 hold
junk covered by zero lhsT columns. otile/psum: [0:52] = even output
rows, [64:116] = odd.

Passthrough mosaic values never touch the TensorEngine: only the 2/3
of outputs needing interpolation go through matmuls, packed as 4 psum
groups per (chunk, cc) -- each group stacks two half-planes (M=116)
sharing the same stride-2 rhs columns:

  grp0 (even px): G@even-rows (K0) + R@odd-rows (K2)
  grp1 (even px): B@even-rows (K3) + B@odd-rows (K1)
  grp2 (odd  px): B@even-rows (K2) + G@odd-rows (K0)
  grp3 (odd  px): R@even-rows (K1) + R@odd-rows (K3)

All matmul operands are bf16 (exact for the dyadic MHC taps; ~2^-9
relative rounding on x, far under the 2e-2 gate). Banded lhsT encode
the 5 vertical taps permuted to the deinterleaved layout; 5
accumulating matmuls (horizontal taps) per group.

Drains: one dual-op scalar_tensor_tensor per half-plane family on DVE
(clip low+high + int32 cast + stride-6 channel-interleaved write; 6
ops per chunk-cc). Passthrough: 4 Copy activations on the otherwise
idle Scalar engine. All DMA issue on gpsimd (fast DGE). Even/odd
output rows stored with separate row-strided DMAs.

_split_waits post-pass: this container's walrus accepts only ONE
semaphore wait per instruction, so excess Tile-emitted waits are
hoisted onto preceding same-engine NOPs (sequencer order preserves
semantics).
"""

import sys

import numpy as np

sys.path.insert(0, "/opt/trn_rl_repo")

import ml_dtypes

H, W = 4096, 6144
NCORES = 8
RB = H // NCORES          # 512 output rows per core
CR = 104                  # output rows per chunk (uniform, overlapping)
HR = CR // 2              # 52 rows per parity half
KR = CR + 4               # 108 window rows per chunk
ODD = 64                  # partition base of the odd-parity block
MM = ODD + HR             # 116 = psum/otile partitions (incl. dead 52..63)
KK = ODD + KR // 2        # 118 = itile partitions
CW = 1024                 # output px per column-chunk
NCC = W // CW             # 6 column-chunks
R0S = (0, 102, 204, 306, 408)   # chunk starts (rows 510..511 overlap)

# (col_parity, (half0_kernel, half0_chan), (half1_kernel, half1_chan))
GRPDEF = (
    (0, (0, 1), (2, 0)),
    (0, (3, 2), (1, 2)),
    (1, (2, 2), (0, 1)),
    (1, (1, 0), (3, 0)),
)
# passthrough: (col_parity, chan, row_parity)
PDEF = ((0, 0, 0), (0, 1, 1), (1, 1, 0), (1, 2, 1))


def _perm(w: int) -> int:
    """Window row w -> deinterleaved itile partition."""
    if w % 2 == 0:
        return 53 if w == 0 else w // 2 - 1
    return ODD + 53 if w == 1 else ODD + (w - 3) // 2


def _build_weights(kernels: np.ndarray) -> np.ndarray:
    """Deinterleave-permuted banded lhsT blocks, (128, 20*MM) bf16."""
    K = kernels[:, 0].astype(np.float32)  # (4,5,5)
    wts = np.zeros((128, 20 * MM), np.float32)
    for g, (_cp, h0, h1) in enumerate(GRPDEF):
        for dxi in range(5):
            blk = (g * 5 + dxi) * MM
            for half, (ki, _ch) in enumerate((h0, h1)):
                for mp in range(HR):
                    col = blk + half * ODD + mp
                    l = 2 * mp + half
                    for dyi in range(5):
                        v = K[ki, dyi, dxi]
                        if v != 0.0:
                            wts[_perm(l + dyi), col] = v
    return wts.astype(ml_dtypes.bfloat16)


def _split_waits(nc, maxw=1):
    """Hoist excess semaphore waits onto preceding same-engine NOPs."""
    import concourse.mybir as mybir

    nsplit = 0
    for f in nc.m.functions:
        for b in f.blocks:
            new = []
            for inst in list(b.instructions):
                si = inst.sync_info
                ow = list(si.on_wait) if si and si.on_wait else []
                if len(ow) > maxw:
                    for wx in ow[:-maxw]:
                        new.append(mybir.InstNoOp(
                            name=inst.name + f"-w{nsplit}",
                            sync_info=mybir.SyncInfo(on_wait=[wx], on_update=[]),
                            engine=inst.engine,
                            bass_nofuse=True,
                        ))
                        nsplit += 1
                    si.on_wait = ow[-maxw:]
                new.append(inst)
            b.instructions = new
    return nsplit


def _build_bass():
    import contextlib

    import concourse.bass as bass
    import concourse.mybir as mybir
    import concourse.tile as tile

    bf16 = mybir.dt.bfloat16
    f32 = mybir.dt.float32
    i32 = mybir.dt.int32
    NP = 512                  # psum cols = output px per (cc, parity)
    OB = CW * 3               # otile cols (3072)

    nc = bass.Bass()
    xb = nc.declare_dram_parameter("xb", [RB + 4, W + 4], bf16, isOutput=False)
    wts = nc.declare_dram_parameter("wts", [128, 20 * MM], bf16, isOutput=False)
    out = nc.declare_dram_parameter("out", [RB, W * 3], i32, isOutput=True)

    with contextlib.ExitStack() as ctx:
        tc = ctx.enter_context(tile.TileContext(nc))
        wpool = ctx.enter_context(tc.tile_pool(name="wpool", bufs=1))
        inpool = ctx.enter_context(tc.tile_pool(name="inpool", bufs=1))
        opool = ctx.enter_context(tc.tile_pool(name="opool", bufs=4))
        pspool = ctx.enter_context(tc.tile_pool(name="pspool", bufs=2,
                                                space="PSUM"))

        wtile = wpool.tile([128, 20 * MM], bf16)
        nc.gpsimd.dma_start(wtile[:], wts[:])
        climit = wpool.tile([128, NP], f32)
        nc.gpsimd.memset(climit[:], 16777215.0)

        # deinterleaved input tiles.  All DMA issue stays on gpsimd's
        # fast DGE queue; the runtime round-robins DMA instructions onto
        # DMA engines, so main loads are split into column quarters to
        # balance transfer time across engines.  Chunks 0-1 are issued
        # up front; chunk g+2 is issued inside chunk g's loop so input
        # issue never head-of-line-blocks the output stores.
        itiles = []
        QW = (W + 4) // 4

        def load_chunk(g):
            r0 = R0S[g]
            it = inpool.tile([KK, W + 4], bf16, tag=f"it{g}", name=f"it{g}")
            for q in range(4):
                c0, c1 = q * QW, (q + 1) * QW
                nc.gpsimd.dma_start(it[0:53, c0:c1],
                                    xb[r0 + 2 : r0 + KR - 1 : 2, c0:c1])
                nc.gpsimd.dma_start(it[64:117, c0:c1],
                                    xb[r0 + 3 : r0 + KR : 2, c0:c1])
            nc.gpsimd.dma_start(it[53:54, :], xb[r0 : r0 + 1, :])
            # partitions 54..63: zero-weight junk; fill with finite data
            nc.gpsimd.dma_start(it[54:64, :], xb[r0 + 2 : r0 + 22 : 2, :])
            nc.gpsimd.dma_start(it[117:118, :], xb[r0 + 1 : r0 + 2, :])
            itiles.append(it)

        load_chunk(0)
        load_chunk(1)

        for g, r0 in enumerate(R0S):
            it = itiles[g]
            for cc in range(NCC):
                if cc == 0 and g + 2 < len(R0S):
                    load_chunk(g + 2)
                otile = opool.tile([MM, OB], i32, tag="otile")
                for gi, (cp, h0, h1) in enumerate(GRPDEF):
                    ptile = pspool.tile([MM, NP], f32, tag=f"ps{gi}")
                    for dxi in range(5):
                        blk = (gi * 5 + dxi) * MM
                        c0 = CW * cc + cp + dxi
                        nc.tensor.matmul(
                            ptile[:, :],
                            wtile[:KK, blk : blk + MM],
                            it[:KK, c0 : c0 + 2 * NP - 1 : 2],
                            start=(dxi == 0),
                            stop=(dxi == 4),
                        )
                    if gi == 3:
                        # fused R@odd-px drain on Scalar engine:
                        # clip(x,0,M) = Relu(M - Relu(M - x))
                        base = 3 * cp + h0[1]
                        stage = opool.tile([MM, NP], f32, tag="stage",
                                           name="stage")
                        nc.scalar.activation(
                            stage[:, :], ptile[:, :],
                            mybir.ActivationFunctionType.Relu,
                            bias=climit[:MM, 0:1], scale=-1.0,
                        )
                        nc.scalar.activation(
                            otile[:, base : OB : 6], stage[:, :],
                            mybir.ActivationFunctionType.Relu,
                            bias=climit[:MM, 0:1], scale=-1.0,
                        )
                    elif h0[1] == h1[1]:  # same channel: one fused drain
                        base = 3 * cp + h0[1]
                        nc.vector.scalar_tensor_tensor(
                            otile[:, base : OB : 6],
                            ptile[:, :], 0.0, climit[:MM, :],
                            op0=mybir.AluOpType.max, op1=mybir.AluOpType.min,
                        )
                    else:
                        for half, (_ki, ch) in enumerate((h0, h1)):
                            base = 3 * cp + ch
                            p0 = half * ODD
                            nc.vector.scalar_tensor_tensor(
                                otile[p0 : p0 + HR, base : OB : 6],
                                ptile[p0 : p0 + HR, :], 0.0,
                                climit[p0 : p0 + HR, :],
                                op0=mybir.AluOpType.max,
                                op1=mybir.AluOpType.min,
                            )
                # passthrough fills: even-row planes on gpsimd (idle),
                # odd-row planes on the Scalar engine
                for cp, ch, rp in PDEF:
                    base = 3 * cp + ch
                    s0 = rp * ODD
                    c0 = 2 + cp + CW * cc
                    if rp == 0:
                        nc.gpsimd.tensor_copy(
                            otile[s0 : s0 + HR, base : OB : 6],
                            it[s0 : s0 + HR, c0 : c0 + 2 * NP - 1 : 2],
                        )
                    else:
                        nc.scalar.activation(
                            otile[s0 : s0 + HR, base : OB : 6],
                            it[s0 : s0 + HR, c0 : c0 + 2 * NP - 1 : 2],
                            mybir.ActivationFunctionType.Copy,
                        )
                # store: even rows then odd rows (row-strided in HBM);
                # the very last otile is split column-wise to cut the tail
                ob0 = OB * cc
                nsp = 4 if (g == len(R0S) - 1 and cc == NCC - 1) else 1
                step = OB // nsp
                for sp in range(nsp):
                    o0 = sp * step
                    nc.gpsimd.dma_start(
                        out[r0 : r0 + CR : 2, ob0 + o0 : ob0 + o0 + step],
                        otile[0:HR, o0 : o0 + step])
                    nc.gpsimd.dma_start(
                        out[r0 + 1 : r0 + CR : 2, ob0 + o0 : ob0 + o0 + step],
                        otile[ODD : ODD + HR, o0 : o0 + step])
    _split_waits(nc)
    return nc


_BASS_CACHE = {}


def _get_nc():
    if "nc" not in _BASS_CACHE:
        _BASS_CACHE["nc"] = _build_bass()
    return _BASS_CACHE["nc"]


def _prepare(x: np.ndarray, kernels: np.ndarray):
    x = np.asarray(x)
    kernels = np.asarray(kernels)
    assert x.shape == (H, W) and x.dtype == np.int32

    xp = np.pad(x, 2, mode="reflect").astype(ml_dtypes.bfloat16)
    wts = _build_weights(kernels)
    in_maps = []
    for c in range(NCORES):
        band = np.ascontiguousarray(xp[c * RB : c * RB + RB + 4, :])
        in_maps.append({"xb": band, "wts": wts})
    return in_maps


def _finish(res) -> np.ndarray:
    parts = [res.results[c]["out"] for c in range(NCORES)]
    full = np.concatenate(parts, axis=0)  # (H, W*3)
    return full.reshape(H, W, 3).astype(np.int32, copy=False)


def kernel(x: np.ndarray, kernels: np.ndarray) -> np.ndarray:
    from concourse.bass_utils import run_bass_kernel_spmd

    in_maps = _prepare(x, kernels)
    nc = _get_nc()
    res = run_bass_kernel_spmd(nc, in_maps, core_ids=list(range(NCORES)))
    return _finish(res)
